# revision 23
# baseline (speedup 1.0000x reference)
"""MultiHeadAttn Trainium2 kernel: 8-core data/sequence-parallel, no collectives.

Layer: post-LN multi-head attention (B=4, S=2048, D=1024, H=16, DH=64), fp32 io.
  q,k,v = h@Wq, h@Wk, h@Wv ; scores = q k^T * 1/8 ; probs = softmax_j
  out = LN(h + (probs v) @ Wo)

Sharding: 8 cores x 1024 query rows (core c: batch c//2, seq-half c%2).
Each core recomputes k/v projections for its batch's full 2048 rows.

Pipeline design (v2): the ScalarE exp stream (256 ACTIVATEs of [128,1024],
~290us) is the hard floor; everything else must hide under it without ever
letting the PE idle >3.4us (HAM re-throttles the PE clock 2.4->1.2 GHz).

  - scores for a head PAIR are packed into one [128kv, 2x512q] PSUM tile via
    K=64 row-tiles (partitions 0-63 / 64-127 stream concurrently), one exp
    ACTIVATE covers both heads.
  - queries are processed in two passes of 512 columns so PSUM fits:
    sc 2x[128,1024] (4 banks) + av 2x[65,512] (2) + proj 2x[128,512] (2).
  - softmax via constant shift exp(s*0.125 - 60); the ones-column on v gives
    denominators in av row 64. Read-out evacuates the raw [65,512] av tile in
    one same-base copy (frees the bank fast); reciprocal (part 64 -> 0),
    gpsimd partition-broadcast and the normalizing multiply run off the
    critical path during the next pair. All DVE ops keep walrus's
    samePartitions rule: multi-input ops have equal input base partitions.
  - k/q projections of the NEXT pair are emitted inside the current pair's
    chunk loop; the ready-first Tile scheduler drops them into the PE's
    exp-wait gaps. V projection interleaves with pair 0.
  - pair-0 k/q projection is contraction-outer so matmuls stream behind the
    hT DMA arrivals; a dummy exp preloads the ACT table during the DMA.
  - o-proj accumulates the residual via an identity matmul (h enters PSUM
    through the PE), so LayerNorm stats read o_ps directly.
"""

import numpy as np
import ml_dtypes

import concourse.bass as bass
import concourse.mybir as mybir
from concourse import bacc
from concourse.tile import TileContext
from concourse.tile_rust import add_dep_helper
from concourse.bass_utils import run_bass_kernel_spmd

B, S, D, H, DH = 4, 2048, 1024, 16, 64
SCALE = 1.0 / (DH ** 0.5)
LN_EPS = 1e-5
EXP_C = 60.0          # constant softmax shift; see baseline notes
N_CORES = 8
SQ = B * S // N_CORES  # 1024 query rows per core
KC = D // 128          # 8 contraction chunks
MC = (H * DH) // 128   # 8 head-pair chunks
SC = S // 128          # 16 kv-sequence chunks
QC = SQ // 128         # 8 query-row chunks (phase C)
QB = 512               # query block per pass
VW = DH + 1            # v columns per head incl. ones column

bf16 = mybir.dt.bfloat16
fp16 = mybir.dt.float16
f32 = mybir.dt.float32

_CACHE: dict = {}


def _build(apply_gb: bool):
    nc = bacc.Bacc("TRN2", target_bir_lowering=False, debug=False)
    hT = nc.dram_tensor("hT", [128, KC, S], fp16, kind="ExternalInput")
    hres = nc.dram_tensor("hres", [128, QC, D], fp16, kind="ExternalInput")
    wq = nc.dram_tensor("wq", [128, KC, D], fp16, kind="ExternalInput")
    wk = nc.dram_tensor("wk", [128, KC, D], fp16, kind="ExternalInput")
    wv = nc.dram_tensor("wv", [128, KC, D], fp16, kind="ExternalInput")
    wo = nc.dram_tensor("wo", [128, KC, D], bf16, kind="ExternalInput")
    ident = nc.dram_tensor("ident", [128, 128], fp16, kind="ExternalInput")
    gb = nc.dram_tensor("gb", [1, 2 * D], f32, kind="ExternalInput")
    out = nc.dram_tensor("out", [128, QC, D], f32, kind="ExternalOutput")

    with TileContext(nc) as tc:
        with (
            tc.tile_pool(name="persist", bufs=1) as persist,
            tc.tile_pool(name="pex", bufs=3) as pex,        # exp output tiles
            tc.tile_pool(name="pavr", bufs=2) as pavr,      # av staging (SBUF)
            tc.tile_pool(name="prec", bufs=1) as prec,      # reciprocals
            tc.tile_pool(name="pbc", bufs=1) as pbc,        # broadcast tiles
            tc.tile_pool(name="paw", bufs=2) as paw,        # streamed weights
            tc.tile_pool(name="psc", bufs=2, space="PSUM") as psc,   # 4 banks
            tc.tile_pool(name="pava", bufs=1, space="PSUM") as pava, # 1 bank
            tc.tile_pool(name="pavb", bufs=1, space="PSUM") as pavb, # 1 bank
            tc.tile_pool(name="ppj", bufs=2, space="PSUM") as ppj,   # 2 banks
        ):
            hT_sb = persist.tile([128, KC, S], fp16)
            kT = persist.tile([128, MC, S], fp16)
            qT = persist.tile([128, MC, SQ], fp16)
            vaug = persist.tile([128, SC, H * VW], bf16)
            avT = persist.tile([128, MC, SQ], bf16)
            ident_sb = persist.tile([128, 128], fp16)
            biasC = persist.tile([128, 1], f32)
            eps_t = persist.tile([128, 1], f32)
            scr = persist.tile([128, 1], f32)
            nc.vector.memset(biasC, -EXP_C)
            nc.vector.memset(eps_t, LN_EPS)
            vv = vaug[:, :, :].rearrange("p c (h x) -> p c h x", x=VW)
            nc.vector.memset(vv[:, :, :, DH:VW], 1.0)
            # preload the exp table set while startup DMAs run
            nc.scalar.activation(
                out=scr[:, :], in_=biasC[:, :],
                func=mybir.ActivationFunctionType.Exp, scale=1.0)

            # PE warmup: ~5us of tiny matmuls while DMAs land, so the HAM
            # clock-gate opens before the first projection stream.
            warm = persist.tile([128, 64], f32)
            nc.vector.memset(warm, 0.0)
            wps = ppj.tile([128, 64], f32, tag="pj", name="warm")
            for _ in range(60):
                nc.tensor.matmul(wps[0:64, :], warm[:, 0:64], warm[:, :],
                                 start=True, stop=True)

            # weight DMAs for pair 0 first, then the h stream
            wk_t0 = paw.tile([128, KC, 128], fp16, tag="wk")
            nc.sync.dma_start(out=wk_t0, in_=wk[:, :, 0:128])
            wq_t0 = paw.tile([128, KC, 128], fp16, tag="wq")
            nc.sync.dma_start(out=wq_t0, in_=wq[:, :, 0:128])
            nc.sync.dma_start(out=ident_sb[:, :], in_=ident[:, :])
            for kc in range(KC):
                (nc.sync if kc % 2 == 0 else nc.scalar).dma_start(
                    out=hT_sb[:, kc, :], in_=hT[:, kc, :])

            def _accum_units(lhs_of_kc, rhs_of_kc, cast_fn, name):
                """One 8-deep matmul accumulation + evacuating cast, emitted
                as single-instruction units so filler interleaves at matmul
                granularity (a whole ready group would outrank later-emitted
                attention ops in the priority heap and starve ScalarE)."""
                state = {}

                def mk_mm(kc):
                    def emit():
                        if "ps" not in state:
                            state["ps"] = ppj.tile([128, 512], f32,
                                                   tag="pj", name=name)
                        nc.tensor.matmul(
                            state["ps"][:, :], lhs_of_kc(kc), rhs_of_kc(kc),
                            start=(kc == 0), stop=(kc == KC - 1),
                        )
                    return emit

                def mk_cast():
                    def emit():
                        cast_fn(state["ps"])
                    return emit
                return [mk_mm(kc) for kc in range(KC)] + [mk_cast()]

            def kproj(mc):
                wk_t = paw.tile([128, KC, 128], fp16, tag="wk")
                nc.sync.dma_start(out=wk_t, in_=wk[:, :, mc * 128:(mc + 1) * 128])
                units = []
                for c4 in range(4):
                    lo = c4 * 512
                    units += _accum_units(
                        lambda kc: wk_t[:, kc, :],
                        lambda kc, lo=lo: hT_sb[:, kc, lo:lo + 512],
                        lambda ps, lo=lo: nc.vector.tensor_copy(
                            out=kT[:, mc, lo:lo + 512], in_=ps[:, :]),
                        "kps")
                return units

            def qproj(mc, qh):
                wq_t = paw.tile([128, KC, 128], fp16, tag="wq")
                nc.sync.dma_start(out=wq_t, in_=wq[:, :, mc * 128:(mc + 1) * 128])
                lo = qh * QB
                return _accum_units(
                    lambda kc: wq_t[:, kc, :],
                    lambda kc: hT_sb[:, kc, lo:lo + QB],
                    lambda ps: nc.vector.tensor_copy(
                        out=qT[:, mc, lo:lo + QB], in_=ps[:, :]),
                    "qps")

            def vchunk(sc):
                units = []
                for n in range(2):
                    units += _accum_units(
                        lambda kc, sc=sc: hT_sb[:, kc, sc * 128:(sc + 1) * 128],
                        lambda kc, n=n: wv_sb[:, kc, n * 512:(n + 1) * 512],
                        lambda ps, sc=sc, n=n: nc.vector.tensor_copy(
                            out=vv[:, sc, n * 8:(n + 1) * 8, 0:DH],
                            in_=ps[:, :].rearrange("p (h x) -> p h x", x=DH)),
                        "vps")
                return units

            last_exp = [None]
            oq_sb = persist.tile([128, 4, D], fp16)
            st_q = persist.tile([128, 4, 2, 6], f32)

            def ochunk_early(q):
                """o-proj + bn_stats for q-chunk q (pass-0 query range), run
                during pass 1 as spread units; LN scale deferred to phase C."""
                units = []
                for ni, n in enumerate(range(0, D, 512)):
                    state = {}

                    def mk(i, q=q, ni=ni, n=n, state=state):
                        def emit():
                            if "ps" not in state:
                                state["ps"] = ppj.tile(
                                    [128, 512], f32, tag="pj", name="oeps")
                            ops = state["ps"]
                            if i < 0:
                                nc.tensor.matmul(
                                    ops[:, :], ident_sb[:, :],
                                    hres_sb[:, q, n:n + 512],
                                    start=True, stop=False)
                            elif i < MC:
                                nc.tensor.matmul(
                                    ops[:, :],
                                    avT[:, i, q * 128:(q + 1) * 128],
                                    wo_sb[:, i, n:n + 512],
                                    start=False, stop=(i == MC - 1))
                            else:
                                nc.vector.bn_stats(
                                    out=st_q[:, q, ni, :], in_=ops[:, :])
                                nc.vector.tensor_copy(
                                    out=oq_sb[:, q, n:n + 512], in_=ops[:, :])
                        return emit
                    units += [mk(i) for i in range(-1, MC + 1)]
                return units

            def attn(mc, qh, units=None):
                """Attention for head pair mc on query block qh.

                units: flat list of single-instruction filler thunks
                (projections for upcoming pairs, V chunks, early o-proj),
                spread round-robin across the 16 kv chunks so the scheduler
                can slot them into the PE's exp-wait gaps without a ready
                group ever starving ScalarE.
                """
                hA, hB = 2 * mc, 2 * mc + 1
                q0 = qh * QB
                rem = list(units or [])
                avA = pava.tile([VW, QB], f32, tag="avA", name="avA")
                avB = pavb.tile([VW, QB], f32, tag="avB", name="avB")
                for sc in range(SC):
                    take = -(-len(rem) // (SC - sc)) if rem else 0
                    for _ in range(take):
                        rem.pop(0)()
                    sc_ps = psc.tile([128, 2 * QB], f32, tag="sc",
                                     name=f"scp{sc % 2}")
                    nc.tensor.matmul(
                        sc_ps[:, 0:QB],
                        kT[0:64, mc, sc * 128:(sc + 1) * 128],
                        qT[0:64, mc, q0:q0 + QB],
                        start=True, stop=True,
                    )
                    nc.tensor.matmul(
                        sc_ps[:, QB:2 * QB],
                        kT[64:128, mc, sc * 128:(sc + 1) * 128],
                        qT[64:128, mc, q0:q0 + QB],
                        start=True, stop=True,
                    )
                    ex = pex.tile([128, 2 * QB], bf16, tag="ex", name="ex")
                    last_exp[0] = nc.scalar.activation(
                        out=ex[:, :], in_=sc_ps[:, :],
                        func=mybir.ActivationFunctionType.Exp,
                        bias=biasC[:, :], scale=SCALE,
                    )
                    nc.tensor.matmul(
                        avA[:, :], vaug[:, sc, hA * VW:(hA + 1) * VW],
                        ex[:, 0:QB],
                        start=(sc == 0), stop=(sc == SC - 1),
                    )
                    nc.tensor.matmul(
                        avB[:, :], vaug[:, sc, hB * VW:(hB + 1) * VW],
                        ex[:, QB:2 * QB],
                        start=(sc == 0), stop=(sc == SC - 1),
                    )
                # read-out: evacuate each [65, QB] av tile in one same-base
                # copy (frees its PSUM bank), then normalize off the critical
                # path: reciprocal of row 64 into partition 0, gpsimd
                # broadcast, multiply with both inputs at base partition 0.
                avRs = []
                for avX, tag in ((avA, "avrA"), (avB, "avrB")):
                    avR = pavr.tile([VW, QB], f32, tag=tag, name=tag)
                    nc.vector.tensor_copy(out=avR[:, :], in_=avX[:, :])
                    avRs.append(avR)
                for i, (avR, po) in enumerate(zip(avRs, (0, 64))):
                    rec = prec.tile([1, QB], f32, tag=f"rec{i}", name="rec")
                    nc.vector.reciprocal(out=rec[:, :], in_=avR[DH:VW, :])
                    bc = pbc.tile([64, QB], f32, tag="bc", name="bc")
                    nc.gpsimd.partition_broadcast(
                        out_ap=bc[:, :], in_ap=rec[0:1, :])
                    nc.vector.tensor_mul(
                        out=avT[po:po + 64, mc, q0:q0 + QB],
                        in0=avR[0:DH, :], in1=bc[:, :],
                    )

            # ---- startup: pair-0 k/q projection streams behind the hT DMA
            # (contraction-outer, accumulating into the two sc-pool tiles) ----
            kA = psc.tile([128, 2 * QB], f32, tag="sc", name="kA")
            kB = psc.tile([128, 2 * QB], f32, tag="sc", name="kB")
            qp = ppj.tile([128, QB], f32, tag="pj", name="qp0")
            for kc in range(KC):
                nc.tensor.matmul(
                    qp[:, :], wq_t0[:, kc, :], hT_sb[:, kc, 0:QB],
                    start=(kc == 0), stop=(kc == KC - 1),
                )
                for c2, t in ((0, kA), (1, kB)):
                    for n in (0, QB):
                        nc.tensor.matmul(
                            t[:, n:n + QB], wk_t0[:, kc, :],
                            hT_sb[:, kc, c2 * 1024 + n:c2 * 1024 + n + QB],
                            start=(kc == 0), stop=(kc == KC - 1),
                        )
            nc.vector.tensor_copy(out=kT[:, 0, 0:1024], in_=kA[:, :])
            nc.vector.tensor_copy(out=qT[:, 0, 0:QB], in_=qp[:, :])
            nc.vector.tensor_copy(out=kT[:, 0, 1024:2048], in_=kB[:, :])

            # ---- Pass 0 (query cols 0:512) + all projections ----
            with tc.tile_pool(name="pav", bufs=1) as pav:
                wv_sb = pav.tile([128, KC, D], fp16)
                nc.gpsimd.dma_start(out=wv_sb[:, :, :], in_=wv[:, :, :])
                u0 = []
                for sc in range(SC):
                    u0 += vchunk(sc)
                u0 += kproj(1) + qproj(1, 0)
                attn(0, 0, units=u0)
                for mc in range(1, MC):
                    if mc < MC - 1:
                        un = kproj(mc + 1) + qproj(mc + 1, 0)
                    else:
                        un = qproj(0, 1)
                    attn(mc, 0, units=un)

            # ---- Pass 1 (query cols 512:1024) ----
            with (
                tc.tile_pool(name="pcw", bufs=1) as pcw,
                tc.tile_pool(name="pc", bufs=2) as pc,
                tc.tile_pool(name="pcs", bufs=2) as pcs,
            ):
                wo_sb = pcw.tile([128, KC, D], bf16)
                nc.sync.dma_start(out=wo_sb[:, :, :], in_=wo[:, :, :])
                hres_sb = pcw.tile([128, QC, D], fp16)
                for q in range(QC):
                    nc.sync.dma_start(out=hres_sb[:, q, :], in_=hres[:, q, :])
                if apply_gb:
                    gb_sb = pcw.tile([128, 2 * D], f32)
                    nc.gpsimd.dma_start(
                        out=gb_sb,
                        in_=bass.AP(tensor=gb, offset=0,
                                    ap=[[0, 128], [1, 2 * D]]),
                    )
                for mc in range(MC):
                    un = []
                    if 2 <= mc <= 5:
                        un += ochunk_early(mc - 2)
                    if mc < MC - 1:
                        un += qproj(mc + 1, 1)
                    attn(mc, 1, units=un or None)

                # ---- Phase C: finish early chunks, then remaining o-proj ----
                def ln_tail(q, stats_ap, x_ap):
                    mv = pcs.tile([128, 2], f32, tag="mv")
                    nc.vector.bn_aggr(out=mv[:, :], in_=stats_ap)
                    rstd = pcs.tile([128, 1], f32, tag="rstd")
                    sq = nc.scalar.activation(
                        out=rstd[:, :], in_=mv[:, 1:2],
                        func=mybir.ActivationFunctionType.Sqrt,
                        bias=eps_t[:, :], scale=1.0,
                    )
                    if last_exp[0] is not None:
                        add_dep_helper(sq.ins, last_exp[0].ins, sync=True,
                                       reason="sqrt after all exps (ACT table)")
                    nc.vector.reciprocal(out=rstd[:, :], in_=rstd[:, :])
                    y = pc.tile([128, D], f32, tag="y")
                    nc.vector.tensor_scalar(
                        out=y[:, :], in0=x_ap,
                        scalar1=mv[:, 0:1], scalar2=rstd[:, :],
                        op0=mybir.AluOpType.subtract,
                        op1=mybir.AluOpType.mult,
                    )
                    if apply_gb:
                        nc.vector.tensor_mul(
                            out=y[:, :], in0=y[:, :], in1=gb_sb[:, 0:D])
                        nc.vector.tensor_add(
                            out=y[:, :], in0=y[:, :], in1=gb_sb[:, D:2 * D])
                    (nc.sync if q % 2 == 0 else nc.scalar).dma_start(
                        out=out[:, q, :], in_=y[:, :])

                for q in range(4):
                    ln_tail(q, st_q[:, q, :, :], oq_sb[:, q, :])
                for q in range(4, QC):
                    o_ps = psc.tile([128, D], f32, tag="sc", name="ops")
                    for n in range(0, D, 512):
                        nc.tensor.matmul(
                            o_ps[:, n:n + 512], ident_sb[:, :],
                            hres_sb[:, q, n:n + 512],
                            start=True, stop=False,
                        )
                        for mc in range(MC):
                            nc.tensor.matmul(
                                o_ps[:, n:n + 512],
                                avT[:, mc, q * 128:(q + 1) * 128],
                                wo_sb[:, mc, n:n + 512],
                                start=False, stop=(mc == MC - 1),
                            )
                    st = pcs.tile([128, 2, 6], f32, tag="st")
                    nc.vector.bn_stats(out=st[:, 0, :], in_=o_ps[:, 0:512])
                    nc.vector.bn_stats(out=st[:, 1, :], in_=o_ps[:, 512:1024])
                    ln_tail(q, st[:, :, :], o_ps[:, :])

    nc.finalize()
    return nc


def _part_major(a: np.ndarray, chunks: int) -> np.ndarray:
    """[chunks*128, N] -> [128, chunks, N] (partition-major device layout)."""
    n = a.shape[1]
    return np.ascontiguousarray(a.reshape(chunks, 128, n).transpose(1, 0, 2))


def kernel(h, Wq, Wk, Wv, Wo, gamma, beta):
    h = np.asarray(h, dtype=np.float32)
    bf = ml_dtypes.bfloat16
    f16 = np.float16
    gamma = np.asarray(gamma, np.float32)
    beta = np.asarray(beta, np.float32)
    apply_gb = not (np.all(gamma == 1.0) and np.all(beta == 0.0))
    wq_d = _part_major(np.asarray(Wq).astype(f16), KC)
    wk_d = _part_major(np.asarray(Wk).astype(f16), KC)
    wv_d = _part_major(np.asarray(Wv).astype(f16), KC)
    wo_d = _part_major(np.asarray(Wo).astype(bf), KC)
    gb = np.concatenate([gamma, beta]).reshape(1, 2 * D)
    ident = np.eye(128, dtype=f16)

    in_maps = []
    for c in range(N_CORES):
        b, r = c // 2, (c % 2) * SQ
        # kv column rotation: this core's queries first (attention is
        # invariant to kv ordering; k/v are projected in the same order)
        hT_b = np.ascontiguousarray(
            np.roll(h[b], -r, axis=0).T).astype(f16)          # [D, S]
        in_maps.append({
            "hT": _part_major(hT_b, KC),
            "hres": _part_major(
                np.ascontiguousarray(h[b, r:r + SQ]).astype(f16), QC),
            "wq": wq_d, "wk": wk_d, "wv": wv_d, "wo": wo_d, "gb": gb,
            "ident": ident,
        })

    key = f"nc{int(apply_gb)}"
    if key not in _CACHE:
        _CACHE[key] = _build(apply_gb)
    res = run_bass_kernel_spmd(_CACHE[key], in_maps, core_ids=list(range(N_CORES)))
    _CACHE["last"] = res

    outp = np.empty((B, S, D), dtype=np.float32)
    for c in range(N_CORES):
        b, r = c // 2, (c % 2) * SQ
        o = res.results[c]["out"]  # [128, QC, D]
        outp[b, r:r + SQ] = o.transpose(1, 0, 2).reshape(SQ, D)
    return outp


# revision 24
# speedup vs baseline: 1.0252x; 1.0252x over previous
"""MultiHeadAttn Trainium2 kernel: 8-core data/sequence-parallel, no collectives.

Layer: post-LN multi-head attention (B=4, S=2048, D=1024, H=16, DH=64), fp32 io.
  q,k,v = h@Wq, h@Wk, h@Wv ; scores = q k^T * 1/8 ; probs = softmax_j
  out = LN(h + (probs v) @ Wo)

Sharding: 8 cores x 1024 query rows (core c: batch c//2, seq-half c%2).
Each core recomputes k/v projections for its batch's full 2048 rows.

Pipeline design (v2): the ScalarE exp stream (256 ACTIVATEs of [128,1024],
~290us) is the hard floor; everything else must hide under it without ever
letting the PE idle >3.4us (HAM re-throttles the PE clock 2.4->1.2 GHz).

  - scores for a head PAIR are packed into one [128kv, 2x512q] PSUM tile via
    K=64 row-tiles (partitions 0-63 / 64-127 stream concurrently), one exp
    ACTIVATE covers both heads.
  - queries are processed in two passes of 512 columns so PSUM fits:
    sc 2x[128,1024] (4 banks) + av 2x[65,512] (2) + proj 2x[128,512] (2).
  - softmax via constant shift exp(s*0.125 - 60); the ones-column on v gives
    denominators in av row 64. Read-out evacuates the raw [65,512] av tile in
    one same-base copy (frees the bank fast); reciprocal (part 64 -> 0),
    gpsimd partition-broadcast and the normalizing multiply run off the
    critical path during the next pair. All DVE ops keep walrus's
    samePartitions rule: multi-input ops have equal input base partitions.
  - k/q projections of the NEXT pair are emitted inside the current pair's
    chunk loop; the ready-first Tile scheduler drops them into the PE's
    exp-wait gaps. V projection interleaves with pair 0.
  - pair-0 k/q projection is contraction-outer so matmuls stream behind the
    hT DMA arrivals; a dummy exp preloads the ACT table during the DMA.
  - o-proj accumulates the residual via an identity matmul (h enters PSUM
    through the PE), so LayerNorm stats read o_ps directly.
"""

import numpy as np
import ml_dtypes

import concourse.bass as bass
import concourse.mybir as mybir
from concourse import bacc
from concourse.tile import TileContext
from concourse.tile_rust import add_dep_helper
from concourse.bass_utils import run_bass_kernel_spmd

B, S, D, H, DH = 4, 2048, 1024, 16, 64
SCALE = 1.0 / (DH ** 0.5)
LN_EPS = 1e-5
EXP_C = 60.0          # constant softmax shift; see baseline notes
N_CORES = 8
SQ = B * S // N_CORES  # 1024 query rows per core
KC = D // 128          # 8 contraction chunks
MC = (H * DH) // 128   # 8 head-pair chunks
SC = S // 128          # 16 kv-sequence chunks
QC = SQ // 128         # 8 query-row chunks (phase C)
QB = 512               # query block per pass
VW = DH + 1            # v columns per head incl. ones column

bf16 = mybir.dt.bfloat16
fp16 = mybir.dt.float16
f32 = mybir.dt.float32

_CACHE: dict = {}


def _build(apply_gb: bool):
    nc = bacc.Bacc("TRN2", target_bir_lowering=False, debug=False)
    hT = nc.dram_tensor("hT", [128, KC, S], fp16, kind="ExternalInput")
    hres = nc.dram_tensor("hres", [128, QC, D], fp16, kind="ExternalInput")
    wq = nc.dram_tensor("wq", [128, KC, D], fp16, kind="ExternalInput")
    wk = nc.dram_tensor("wk", [128, KC, D], fp16, kind="ExternalInput")
    wv = nc.dram_tensor("wv", [128, KC, D], fp16, kind="ExternalInput")
    wo = nc.dram_tensor("wo", [128, KC, D], bf16, kind="ExternalInput")
    ident = nc.dram_tensor("ident", [128, 128], fp16, kind="ExternalInput")
    gb = nc.dram_tensor("gb", [1, 2 * D], f32, kind="ExternalInput")
    out = nc.dram_tensor("out", [128, QC, D], f32, kind="ExternalOutput")

    with TileContext(nc) as tc:
        with (
            tc.tile_pool(name="persist", bufs=1) as persist,
            tc.tile_pool(name="pex", bufs=3) as pex,        # exp output tiles
            tc.tile_pool(name="pavr", bufs=2) as pavr,      # av staging (SBUF)
            tc.tile_pool(name="prec", bufs=1) as prec,      # reciprocals
            tc.tile_pool(name="pbc", bufs=1) as pbc,        # broadcast tiles
            tc.tile_pool(name="paw", bufs=2) as paw,        # streamed weights
            tc.tile_pool(name="psc", bufs=2, space="PSUM") as psc,   # 4 banks
            tc.tile_pool(name="pava", bufs=1, space="PSUM") as pava, # 1 bank
            tc.tile_pool(name="pavb", bufs=1, space="PSUM") as pavb, # 1 bank
            tc.tile_pool(name="ppj", bufs=2, space="PSUM") as ppj,   # 2 banks
        ):
            hT_sb = persist.tile([128, KC, S], fp16)
            kT = persist.tile([128, MC, S], fp16)
            qT = persist.tile([128, MC, SQ], fp16)
            vaug = persist.tile([128, SC, H * VW], bf16)
            avT = persist.tile([128, MC, SQ], bf16)
            ident_sb = persist.tile([128, 128], fp16)
            biasC = persist.tile([128, 1], f32)
            eps_t = persist.tile([128, 1], f32)
            scr = persist.tile([128, 1], f32)
            nc.vector.memset(biasC, -EXP_C)
            nc.vector.memset(eps_t, LN_EPS)
            vv = vaug[:, :, :].rearrange("p c (h x) -> p c h x", x=VW)
            nc.vector.memset(vv[:, :, :, DH:VW], 1.0)
            # preload the exp table set while startup DMAs run
            nc.scalar.activation(
                out=scr[:, :], in_=biasC[:, :],
                func=mybir.ActivationFunctionType.Exp, scale=1.0)

            # PE warmup: ~5us of tiny matmuls while DMAs land, so the HAM
            # clock-gate opens before the first projection stream.
            warm = persist.tile([128, 64], f32)
            nc.vector.memset(warm, 0.0)
            wps = ppj.tile([128, 64], f32, tag="pj", name="warm")
            for _ in range(60):
                nc.tensor.matmul(wps[0:64, :], warm[:, 0:64], warm[:, :],
                                 start=True, stop=True)

            # weight DMAs for pair 0 first, then the h stream
            wk_t0 = paw.tile([128, KC, 128], fp16, tag="wk")
            nc.sync.dma_start(out=wk_t0, in_=wk[:, :, 0:128])
            wq_t0 = paw.tile([128, KC, 128], fp16, tag="wq")
            nc.sync.dma_start(out=wq_t0, in_=wq[:, :, 0:128])
            nc.sync.dma_start(out=ident_sb[:, :], in_=ident[:, :])
            for kc in range(KC):
                nc.sync.dma_start(out=hT_sb[:, kc, :], in_=hT[:, kc, :])

            def _accum_units(lhs_of_kc, rhs_of_kc, cast_fn, name):
                """One 8-deep matmul accumulation + evacuating cast, emitted
                as single-instruction units so filler interleaves at matmul
                granularity (a whole ready group would outrank later-emitted
                attention ops in the priority heap and starve ScalarE)."""
                state = {}

                def mk_mm(kc):
                    def emit():
                        if "ps" not in state:
                            state["ps"] = ppj.tile([128, 512], f32,
                                                   tag="pj", name=name)
                        nc.tensor.matmul(
                            state["ps"][:, :], lhs_of_kc(kc), rhs_of_kc(kc),
                            start=(kc == 0), stop=(kc == KC - 1),
                        )
                    return emit

                def mk_cast():
                    def emit():
                        cast_fn(state["ps"])
                    return emit
                return [mk_mm(kc) for kc in range(KC)] + [mk_cast()]

            def kproj(mc):
                wk_t = paw.tile([128, KC, 128], fp16, tag="wk")
                nc.sync.dma_start(out=wk_t, in_=wk[:, :, mc * 128:(mc + 1) * 128])
                units = []
                for c4 in range(4):
                    lo = c4 * 512
                    units += _accum_units(
                        lambda kc: wk_t[:, kc, :],
                        lambda kc, lo=lo: hT_sb[:, kc, lo:lo + 512],
                        lambda ps, lo=lo: nc.vector.tensor_copy(
                            out=kT[:, mc, lo:lo + 512], in_=ps[:, :]),
                        "kps")
                return units

            def qproj(mc, qh):
                wq_t = paw.tile([128, KC, 128], fp16, tag="wq")
                nc.sync.dma_start(out=wq_t, in_=wq[:, :, mc * 128:(mc + 1) * 128])
                lo = qh * QB
                return _accum_units(
                    lambda kc: wq_t[:, kc, :],
                    lambda kc: hT_sb[:, kc, lo:lo + QB],
                    lambda ps: nc.vector.tensor_copy(
                        out=qT[:, mc, lo:lo + QB], in_=ps[:, :]),
                    "qps")

            def vchunk(sc):
                units = []
                for n in range(2):
                    units += _accum_units(
                        lambda kc, sc=sc: hT_sb[:, kc, sc * 128:(sc + 1) * 128],
                        lambda kc, n=n: wv_sb[:, kc, n * 512:(n + 1) * 512],
                        lambda ps, sc=sc, n=n: nc.vector.tensor_copy(
                            out=vv[:, sc, n * 8:(n + 1) * 8, 0:DH],
                            in_=ps[:, :].rearrange("p (h x) -> p h x", x=DH)),
                        "vps")
                return units

            last_exp = [None]
            oq_sb = persist.tile([128, 4, D], fp16)
            st_q = persist.tile([128, 4, 2, 6], f32)

            def ochunk_early(q):
                """o-proj + bn_stats for q-chunk q (pass-0 query range), run
                during pass 1 as spread units; LN scale deferred to phase C."""
                units = []
                for ni, n in enumerate(range(0, D, 512)):
                    state = {}

                    def mk(i, q=q, ni=ni, n=n, state=state):
                        def emit():
                            if "ps" not in state:
                                state["ps"] = ppj.tile(
                                    [128, 512], f32, tag="pj", name="oeps")
                            ops = state["ps"]
                            if i < 0:
                                nc.tensor.matmul(
                                    ops[:, :], ident_sb[:, :],
                                    hres_sb[:, q, n:n + 512],
                                    start=True, stop=False)
                            elif i < MC:
                                nc.tensor.matmul(
                                    ops[:, :],
                                    avT[:, i, q * 128:(q + 1) * 128],
                                    wo_sb[:, i, n:n + 512],
                                    start=False, stop=(i == MC - 1))
                            else:
                                nc.vector.bn_stats(
                                    out=st_q[:, q, ni, :], in_=ops[:, :])
                                nc.vector.tensor_copy(
                                    out=oq_sb[:, q, n:n + 512], in_=ops[:, :])
                        return emit
                    units += [mk(i) for i in range(-1, MC + 1)]
                return units

            def attn(mc, qh, units=None):
                """Attention for head pair mc on query block qh.

                units: flat list of single-instruction filler thunks
                (projections for upcoming pairs, V chunks, early o-proj),
                spread round-robin across the 16 kv chunks so the scheduler
                can slot them into the PE's exp-wait gaps without a ready
                group ever starving ScalarE.
                """
                hA, hB = 2 * mc, 2 * mc + 1
                q0 = qh * QB
                rem = list(units or [])
                avA = pava.tile([VW, QB], f32, tag="avA", name="avA")
                avB = pavb.tile([VW, QB], f32, tag="avB", name="avB")
                for sc in range(SC):
                    take = -(-len(rem) // (SC - sc)) if rem else 0
                    for _ in range(take):
                        rem.pop(0)()
                    sc_ps = psc.tile([128, 2 * QB], f32, tag="sc",
                                     name=f"scp{sc % 2}")
                    nc.tensor.matmul(
                        sc_ps[:, 0:QB],
                        kT[0:64, mc, sc * 128:(sc + 1) * 128],
                        qT[0:64, mc, q0:q0 + QB],
                        start=True, stop=True,
                    )
                    nc.tensor.matmul(
                        sc_ps[:, QB:2 * QB],
                        kT[64:128, mc, sc * 128:(sc + 1) * 128],
                        qT[64:128, mc, q0:q0 + QB],
                        start=True, stop=True,
                    )
                    ex = pex.tile([128, 2 * QB], bf16, tag="ex", name="ex")
                    last_exp[0] = nc.scalar.activation(
                        out=ex[:, :], in_=sc_ps[:, :],
                        func=mybir.ActivationFunctionType.Exp,
                        bias=biasC[:, :], scale=SCALE,
                    )
                    nc.tensor.matmul(
                        avA[:, :], vaug[:, sc, hA * VW:(hA + 1) * VW],
                        ex[:, 0:QB],
                        start=(sc == 0), stop=(sc == SC - 1),
                    )
                    nc.tensor.matmul(
                        avB[:, :], vaug[:, sc, hB * VW:(hB + 1) * VW],
                        ex[:, QB:2 * QB],
                        start=(sc == 0), stop=(sc == SC - 1),
                    )
                # read-out: evacuate each [65, QB] av tile in one same-base
                # copy (frees its PSUM bank), then normalize off the critical
                # path: reciprocal of row 64 into partition 0, gpsimd
                # broadcast, multiply with both inputs at base partition 0.
                avRs = []
                for avX, tag in ((avA, "avrA"), (avB, "avrB")):
                    avR = pavr.tile([VW, QB], f32, tag=tag, name=tag)
                    nc.vector.tensor_copy(out=avR[:, :], in_=avX[:, :])
                    avRs.append(avR)
                for i, (avR, po) in enumerate(zip(avRs, (0, 64))):
                    rec = prec.tile([1, QB], f32, tag=f"rec{i}", name="rec")
                    nc.vector.reciprocal(out=rec[:, :], in_=avR[DH:VW, :])
                    bc = pbc.tile([64, QB], f32, tag="bc", name="bc")
                    nc.gpsimd.partition_broadcast(
                        out_ap=bc[:, :], in_ap=rec[0:1, :])
                    nc.vector.tensor_mul(
                        out=avT[po:po + 64, mc, q0:q0 + QB],
                        in0=avR[0:DH, :], in1=bc[:, :],
                    )

            # ---- startup: pair-0 k/q projection streams behind the hT DMA
            # (contraction-outer, accumulating into the two sc-pool tiles) ----
            kA = psc.tile([128, 2 * QB], f32, tag="sc", name="kA")
            kB = psc.tile([128, 2 * QB], f32, tag="sc", name="kB")
            qp = ppj.tile([128, QB], f32, tag="pj", name="qp0")
            for kc in range(KC):
                nc.tensor.matmul(
                    qp[:, :], wq_t0[:, kc, :], hT_sb[:, kc, 0:QB],
                    start=(kc == 0), stop=(kc == KC - 1),
                )
                for c2, t in ((0, kA), (1, kB)):
                    for n in (0, QB):
                        nc.tensor.matmul(
                            t[:, n:n + QB], wk_t0[:, kc, :],
                            hT_sb[:, kc, c2 * 1024 + n:c2 * 1024 + n + QB],
                            start=(kc == 0), stop=(kc == KC - 1),
                        )
            nc.vector.tensor_copy(out=kT[:, 0, 0:1024], in_=kA[:, :])
            nc.vector.tensor_copy(out=qT[:, 0, 0:QB], in_=qp[:, :])
            nc.vector.tensor_copy(out=kT[:, 0, 1024:2048], in_=kB[:, :])

            # ---- Pass 0 (query cols 0:512) + all projections ----
            with tc.tile_pool(name="pav", bufs=1) as pav:
                wv_sb = pav.tile([128, KC, D], fp16)
                nc.sync.dma_start(out=wv_sb[:, :, :], in_=wv[:, :, :])
                u0 = []
                for sc in range(SC):
                    u0 += vchunk(sc)
                u0 += kproj(1) + qproj(1, 0)
                attn(0, 0, units=u0)
                for mc in range(1, MC):
                    if mc < MC - 1:
                        un = kproj(mc + 1) + qproj(mc + 1, 0)
                    else:
                        un = qproj(0, 1)
                    attn(mc, 0, units=un)

            # ---- Pass 1 (query cols 512:1024) ----
            with (
                tc.tile_pool(name="pcw", bufs=1) as pcw,
                tc.tile_pool(name="pc", bufs=2) as pc,
                tc.tile_pool(name="pcs", bufs=2) as pcs,
            ):
                wo_sb = pcw.tile([128, KC, D], bf16)
                nc.sync.dma_start(out=wo_sb[:, :, :], in_=wo[:, :, :])
                hres_sb = pcw.tile([128, QC, D], fp16)
                for q in range(QC):
                    nc.sync.dma_start(out=hres_sb[:, q, :], in_=hres[:, q, :])
                if apply_gb:
                    gb_sb = pcw.tile([128, 2 * D], f32)
                    nc.gpsimd.dma_start(
                        out=gb_sb,
                        in_=bass.AP(tensor=gb, offset=0,
                                    ap=[[0, 128], [1, 2 * D]]),
                    )
                for mc in range(MC):
                    un = []
                    if 2 <= mc <= 5:
                        un += ochunk_early(mc - 2)
                    if mc < MC - 1:
                        un += qproj(mc + 1, 1)
                    attn(mc, 1, units=un or None)

                # ---- Phase C: finish early chunks, then remaining o-proj ----
                def ln_tail(q, stats_ap, x_ap):
                    mv = pcs.tile([128, 2], f32, tag="mv")
                    nc.vector.bn_aggr(out=mv[:, :], in_=stats_ap)
                    rstd = pcs.tile([128, 1], f32, tag="rstd")
                    sq = nc.scalar.activation(
                        out=rstd[:, :], in_=mv[:, 1:2],
                        func=mybir.ActivationFunctionType.Sqrt,
                        bias=eps_t[:, :], scale=1.0,
                    )
                    if last_exp[0] is not None:
                        add_dep_helper(sq.ins, last_exp[0].ins, sync=True,
                                       reason="sqrt after all exps (ACT table)")
                    nc.vector.reciprocal(out=rstd[:, :], in_=rstd[:, :])
                    y = pc.tile([128, D], f32, tag="y")
                    nc.vector.tensor_scalar(
                        out=y[:, :], in0=x_ap,
                        scalar1=mv[:, 0:1], scalar2=rstd[:, :],
                        op0=mybir.AluOpType.subtract,
                        op1=mybir.AluOpType.mult,
                    )
                    if apply_gb:
                        nc.vector.tensor_mul(
                            out=y[:, :], in0=y[:, :], in1=gb_sb[:, 0:D])
                        nc.vector.tensor_add(
                            out=y[:, :], in0=y[:, :], in1=gb_sb[:, D:2 * D])
                    (nc.sync if q % 2 == 0 else nc.scalar).dma_start(
                        out=out[:, q, :], in_=y[:, :])

                for q in range(4):
                    ln_tail(q, st_q[:, q, :, :], oq_sb[:, q, :])
                for q in range(4, QC):
                    o_ps = psc.tile([128, D], f32, tag="sc", name="ops")
                    for n in range(0, D, 512):
                        nc.tensor.matmul(
                            o_ps[:, n:n + 512], ident_sb[:, :],
                            hres_sb[:, q, n:n + 512],
                            start=True, stop=False,
                        )
                        for mc in range(MC):
                            nc.tensor.matmul(
                                o_ps[:, n:n + 512],
                                avT[:, mc, q * 128:(q + 1) * 128],
                                wo_sb[:, mc, n:n + 512],
                                start=False, stop=(mc == MC - 1),
                            )
                    st = pcs.tile([128, 2, 6], f32, tag="st")
                    nc.vector.bn_stats(out=st[:, 0, :], in_=o_ps[:, 0:512])
                    nc.vector.bn_stats(out=st[:, 1, :], in_=o_ps[:, 512:1024])
                    ln_tail(q, st[:, :, :], o_ps[:, :])

    nc.finalize()
    return nc


def _part_major(a: np.ndarray, chunks: int) -> np.ndarray:
    """[chunks*128, N] -> [128, chunks, N] (partition-major device layout)."""
    n = a.shape[1]
    return np.ascontiguousarray(a.reshape(chunks, 128, n).transpose(1, 0, 2))


def kernel(h, Wq, Wk, Wv, Wo, gamma, beta):
    h = np.asarray(h, dtype=np.float32)
    bf = ml_dtypes.bfloat16
    f16 = np.float16
    gamma = np.asarray(gamma, np.float32)
    beta = np.asarray(beta, np.float32)
    apply_gb = not (np.all(gamma == 1.0) and np.all(beta == 0.0))
    wq_d = _part_major(np.asarray(Wq).astype(f16), KC)
    wk_d = _part_major(np.asarray(Wk).astype(f16), KC)
    wv_d = _part_major(np.asarray(Wv).astype(f16), KC)
    wo_d = _part_major(np.asarray(Wo).astype(bf), KC)
    gb = np.concatenate([gamma, beta]).reshape(1, 2 * D)
    ident = np.eye(128, dtype=f16)

    in_maps = []
    for c in range(N_CORES):
        b, r = c // 2, (c % 2) * SQ
        # kv column rotation: this core's queries first (attention is
        # invariant to kv ordering; k/v are projected in the same order)
        hT_b = np.ascontiguousarray(
            np.roll(h[b], -r, axis=0).T).astype(f16)          # [D, S]
        in_maps.append({
            "hT": _part_major(hT_b, KC),
            "hres": _part_major(
                np.ascontiguousarray(h[b, r:r + SQ]).astype(f16), QC),
            "wq": wq_d, "wk": wk_d, "wv": wv_d, "wo": wo_d, "gb": gb,
            "ident": ident,
        })

    key = f"nc{int(apply_gb)}"
    if key not in _CACHE:
        _CACHE[key] = _build(apply_gb)
    res = run_bass_kernel_spmd(_CACHE[key], in_maps, core_ids=list(range(N_CORES)))
    _CACHE["last"] = res

    outp = np.empty((B, S, D), dtype=np.float32)
    for c in range(N_CORES):
        b, r = c // 2, (c % 2) * SQ
        o = res.results[c]["out"]  # [128, QC, D]
        outp[b, r:r + SQ] = o.transpose(1, 0, 2).reshape(SQ, D)
    return outp


# revision 25
# speedup vs baseline: 1.0289x; 1.0036x over previous
"""MultiHeadAttn Trainium2 kernel: 8-core data/sequence-parallel, no collectives.

Layer: post-LN multi-head attention (B=4, S=2048, D=1024, H=16, DH=64), fp32 io.
  q,k,v = h@Wq, h@Wk, h@Wv ; scores = q k^T * 1/8 ; probs = softmax_j
  out = LN(h + (probs v) @ Wo)

Sharding: 8 cores x 1024 query rows (core c: batch c//2, seq-half c%2).
Each core recomputes k/v projections for its batch's full 2048 rows.

Pipeline design (~468us HW, vs 737us baseline): the ScalarE exp stream
(256 ACTIVATEs of [128,1024], ~280us) is the hard floor; everything else
must hide under it without ever letting the PE idle >3.4us (HAM
re-throttles the PE clock 2.4->1.2 GHz). Pass 1 runs at the exp-stream
floor (~1.11us/chunk); pass 0 is PE-bound by the duplicated K/V
projections (a 2-rank collective exchange would cost more than it saves
at ~35 GB/s).

  - scores for a head PAIR are packed into one [128kv, 2x512q] PSUM tile via
    K=64 row-tiles (partitions 0-63 / 64-127 stream concurrently), one exp
    ACTIVATE covers both heads.
  - queries are processed in two passes of 512 columns so PSUM fits:
    sc 2x[128,1024] (4 banks) + av 2x[65,512] (2) + proj 2x[128,512] (2).
  - softmax via constant shift exp(s*0.125 - 60); the ones-column on v gives
    denominators in av row 64. Read-out evacuates the raw [65,512] av tile in
    one same-base copy (frees the bank fast); reciprocal (part 64 -> 0),
    gpsimd partition-broadcast and the normalizing multiply run off the
    critical path during the next pair. All DVE ops keep walrus's
    samePartitions rule: multi-input ops have equal input base partitions.
  - k/q projections of the NEXT pair, V-projection chunks (pair 0) and
    early o-proj (q-chunks 0-3, during pass 1) are emitted as
    single-matmul filler units spread round-robin across the chunk loop;
    the ready-first Tile scheduler drops them into the PE's exp-wait gaps
    (whole ready groups would outrank later-emitted attention ops and
    starve ScalarE). LN sqrt ops are pinned behind the last exp via
    add_dep_helper so the ACT table never thrashes mid-stream.
  - pair-0 k/q projection is contraction-outer so matmuls stream behind the
    hT DMA arrivals; a dummy exp preloads the ACT table during the DMA.
  - o-proj accumulates the residual via an identity matmul (h enters PSUM
    through the PE), so LayerNorm stats read o_ps directly.
"""

import numpy as np
import ml_dtypes

import concourse.bass as bass
import concourse.mybir as mybir
from concourse import bacc
from concourse.tile import TileContext
from concourse.tile_rust import add_dep_helper
from concourse.bass_utils import run_bass_kernel_spmd

B, S, D, H, DH = 4, 2048, 1024, 16, 64
SCALE = 1.0 / (DH ** 0.5)
LN_EPS = 1e-5
EXP_C = 60.0          # constant softmax shift; see baseline notes
N_CORES = 8
SQ = B * S // N_CORES  # 1024 query rows per core
KC = D // 128          # 8 contraction chunks
MC = (H * DH) // 128   # 8 head-pair chunks
SC = S // 128          # 16 kv-sequence chunks
QC = SQ // 128         # 8 query-row chunks (phase C)
QB = 512               # query block per pass
VW = DH + 1            # v columns per head incl. ones column

bf16 = mybir.dt.bfloat16
fp16 = mybir.dt.float16
f32 = mybir.dt.float32

_CACHE: dict = {}


def _build(apply_gb: bool):
    nc = bacc.Bacc("TRN2", target_bir_lowering=False, debug=False)
    hT = nc.dram_tensor("hT", [128, KC, S], fp16, kind="ExternalInput")
    hres = nc.dram_tensor("hres", [128, QC, D], fp16, kind="ExternalInput")
    wq = nc.dram_tensor("wq", [128, KC, D], fp16, kind="ExternalInput")
    wk = nc.dram_tensor("wk", [128, KC, D], fp16, kind="ExternalInput")
    wv = nc.dram_tensor("wv", [128, KC, D], fp16, kind="ExternalInput")
    wo = nc.dram_tensor("wo", [128, KC, D], bf16, kind="ExternalInput")
    ident = nc.dram_tensor("ident", [128, 128], fp16, kind="ExternalInput")
    gb = nc.dram_tensor("gb", [1, 2 * D], f32, kind="ExternalInput")
    out = nc.dram_tensor("out", [128, QC, D], f32, kind="ExternalOutput")

    with TileContext(nc) as tc:
        with (
            tc.tile_pool(name="persist", bufs=1) as persist,
            tc.tile_pool(name="pex", bufs=3) as pex,        # exp output tiles
            tc.tile_pool(name="pavr", bufs=2) as pavr,      # av staging (SBUF)
            tc.tile_pool(name="prec", bufs=1) as prec,      # reciprocals
            tc.tile_pool(name="pbc", bufs=1) as pbc,        # broadcast tiles
            tc.tile_pool(name="paw", bufs=2) as paw,        # streamed weights
            tc.tile_pool(name="psc", bufs=2, space="PSUM") as psc,   # 4 banks
            tc.tile_pool(name="pava", bufs=1, space="PSUM") as pava, # 1 bank
            tc.tile_pool(name="pavb", bufs=1, space="PSUM") as pavb, # 1 bank
            tc.tile_pool(name="ppj", bufs=2, space="PSUM") as ppj,   # 2 banks
        ):
            hT_sb = persist.tile([128, KC, S], fp16)
            kT = persist.tile([128, MC, S], fp16)
            qT = persist.tile([128, MC, SQ], fp16)
            vaug = persist.tile([128, SC, H * VW], bf16)
            avT = persist.tile([128, MC, SQ], bf16)
            ident_sb = persist.tile([128, 128], fp16)
            biasC = persist.tile([128, 1], f32)
            eps_t = persist.tile([128, 1], f32)
            scr = persist.tile([128, 1], f32)
            nc.vector.memset(biasC, -EXP_C)
            nc.vector.memset(eps_t, LN_EPS)
            vv = vaug[:, :, :].rearrange("p c (h x) -> p c h x", x=VW)
            nc.vector.memset(vv[:, :, :, DH:VW], 1.0)
            # preload the exp table set while startup DMAs run
            nc.scalar.activation(
                out=scr[:, :], in_=biasC[:, :],
                func=mybir.ActivationFunctionType.Exp, scale=1.0)

            # PE warmup: ~5us of tiny matmuls while DMAs land, so the HAM
            # clock-gate opens before the first projection stream.
            warm = persist.tile([128, 64], f32)
            nc.vector.memset(warm, 0.0)
            wps = ppj.tile([128, 64], f32, tag="pj", name="warm")
            for _ in range(60):
                nc.tensor.matmul(wps[0:64, :], warm[:, 0:64], warm[:, :],
                                 start=True, stop=True)

            # weight DMAs for pair 0 first, then the h stream
            wk_t0 = paw.tile([128, KC, 128], fp16, tag="wk")
            nc.sync.dma_start(out=wk_t0, in_=wk[:, :, 0:128])
            wq_t0 = paw.tile([128, KC, 128], fp16, tag="wq")
            nc.sync.dma_start(out=wq_t0, in_=wq[:, :, 0:128])
            nc.sync.dma_start(out=ident_sb[:, :], in_=ident[:, :])
            for kc in range(KC):
                nc.sync.dma_start(out=hT_sb[:, kc, :], in_=hT[:, kc, :])

            def _accum_units(lhs_of_kc, rhs_of_kc, cast_fn, name):
                """One 8-deep matmul accumulation + evacuating cast, emitted
                as single-instruction units so filler interleaves at matmul
                granularity (a whole ready group would outrank later-emitted
                attention ops in the priority heap and starve ScalarE)."""
                state = {}

                def mk_mm(kc):
                    def emit():
                        if "ps" not in state:
                            state["ps"] = ppj.tile([128, 512], f32,
                                                   tag="pj", name=name)
                        nc.tensor.matmul(
                            state["ps"][:, :], lhs_of_kc(kc), rhs_of_kc(kc),
                            start=(kc == 0), stop=(kc == KC - 1),
                        )
                    return emit

                def mk_cast():
                    def emit():
                        cast_fn(state["ps"])
                    return emit
                return [mk_mm(kc) for kc in range(KC)] + [mk_cast()]

            def kproj(mc):
                wk_t = paw.tile([128, KC, 128], fp16, tag="wk")
                nc.sync.dma_start(out=wk_t, in_=wk[:, :, mc * 128:(mc + 1) * 128])
                units = []
                for c4 in range(4):
                    lo = c4 * 512
                    units += _accum_units(
                        lambda kc: wk_t[:, kc, :],
                        lambda kc, lo=lo: hT_sb[:, kc, lo:lo + 512],
                        lambda ps, lo=lo: nc.vector.tensor_copy(
                            out=kT[:, mc, lo:lo + 512], in_=ps[:, :]),
                        "kps")
                return units

            def qproj(mc, qh):
                wq_t = paw.tile([128, KC, 128], fp16, tag="wq")
                nc.sync.dma_start(out=wq_t, in_=wq[:, :, mc * 128:(mc + 1) * 128])
                lo = qh * QB
                return _accum_units(
                    lambda kc: wq_t[:, kc, :],
                    lambda kc: hT_sb[:, kc, lo:lo + QB],
                    lambda ps: nc.vector.tensor_copy(
                        out=qT[:, mc, lo:lo + QB], in_=ps[:, :]),
                    "qps")

            def vchunk(sc):
                units = []
                for n in range(2):
                    units += _accum_units(
                        lambda kc, sc=sc: hT_sb[:, kc, sc * 128:(sc + 1) * 128],
                        lambda kc, n=n: wv_sb[:, kc, n * 512:(n + 1) * 512],
                        lambda ps, sc=sc, n=n: nc.vector.tensor_copy(
                            out=vv[:, sc, n * 8:(n + 1) * 8, 0:DH],
                            in_=ps[:, :].rearrange("p (h x) -> p h x", x=DH)),
                        "vps")
                return units

            last_exp = [None]
            oq_sb = persist.tile([128, 4, D], fp16)
            st_q = persist.tile([128, 4, 2, 6], f32)

            def ochunk_early(q):
                """o-proj + bn_stats for q-chunk q (pass-0 query range), run
                during pass 1 as spread units; LN scale deferred to phase C."""
                units = []
                for ni, n in enumerate(range(0, D, 512)):
                    state = {}

                    def mk(i, q=q, ni=ni, n=n, state=state):
                        def emit():
                            if "ps" not in state:
                                state["ps"] = ppj.tile(
                                    [128, 512], f32, tag="pj", name="oeps")
                            ops = state["ps"]
                            if i < 0:
                                nc.tensor.matmul(
                                    ops[:, :], ident_sb[:, :],
                                    hres_sb[:, q, n:n + 512],
                                    start=True, stop=False)
                            elif i < MC:
                                nc.tensor.matmul(
                                    ops[:, :],
                                    avT[:, i, q * 128:(q + 1) * 128],
                                    wo_sb[:, i, n:n + 512],
                                    start=False, stop=(i == MC - 1))
                            else:
                                nc.vector.bn_stats(
                                    out=st_q[:, q, ni, :], in_=ops[:, :])
                                nc.vector.tensor_copy(
                                    out=oq_sb[:, q, n:n + 512], in_=ops[:, :])
                        return emit
                    units += [mk(i) for i in range(-1, MC + 1)]
                return units

            def attn(mc, qh, units=None):
                """Attention for head pair mc on query block qh.

                units: flat list of single-instruction filler thunks
                (projections for upcoming pairs, V chunks, early o-proj),
                spread round-robin across the 16 kv chunks so the scheduler
                can slot them into the PE's exp-wait gaps without a ready
                group ever starving ScalarE.
                """
                hA, hB = 2 * mc, 2 * mc + 1
                q0 = qh * QB
                rem = list(units or [])
                avA = pava.tile([VW, QB], f32, tag="avA", name="avA")
                avB = pavb.tile([VW, QB], f32, tag="avB", name="avB")
                for sc in range(SC):
                    take = -(-len(rem) // (SC - sc)) if rem else 0
                    for _ in range(take):
                        rem.pop(0)()
                    sc_ps = psc.tile([128, 2 * QB], f32, tag="sc",
                                     name=f"scp{sc % 2}")
                    nc.tensor.matmul(
                        sc_ps[:, 0:QB],
                        kT[0:64, mc, sc * 128:(sc + 1) * 128],
                        qT[0:64, mc, q0:q0 + QB],
                        start=True, stop=True,
                    )
                    nc.tensor.matmul(
                        sc_ps[:, QB:2 * QB],
                        kT[64:128, mc, sc * 128:(sc + 1) * 128],
                        qT[64:128, mc, q0:q0 + QB],
                        start=True, stop=True,
                    )
                    ex = pex.tile([128, 2 * QB], bf16, tag="ex", name="ex")
                    last_exp[0] = nc.scalar.activation(
                        out=ex[:, :], in_=sc_ps[:, :],
                        func=mybir.ActivationFunctionType.Exp,
                        bias=biasC[:, :], scale=SCALE,
                    )
                    nc.tensor.matmul(
                        avA[:, :], vaug[:, sc, hA * VW:(hA + 1) * VW],
                        ex[:, 0:QB],
                        start=(sc == 0), stop=(sc == SC - 1),
                    )
                    nc.tensor.matmul(
                        avB[:, :], vaug[:, sc, hB * VW:(hB + 1) * VW],
                        ex[:, QB:2 * QB],
                        start=(sc == 0), stop=(sc == SC - 1),
                    )
                # read-out: evacuate each [65, QB] av tile in one same-base
                # copy (frees its PSUM bank), then normalize off the critical
                # path: reciprocal of row 64 into partition 0, gpsimd
                # broadcast, multiply with both inputs at base partition 0.
                avRs = []
                for avX, tag in ((avA, "avrA"), (avB, "avrB")):
                    avR = pavr.tile([VW, QB], f32, tag=tag, name=tag)
                    nc.vector.tensor_copy(out=avR[:, :], in_=avX[:, :])
                    avRs.append(avR)
                for i, (avR, po) in enumerate(zip(avRs, (0, 64))):
                    rec = prec.tile([1, QB], f32, tag=f"rec{i}", name="rec")
                    nc.vector.reciprocal(out=rec[:, :], in_=avR[DH:VW, :])
                    bc = pbc.tile([64, QB], f32, tag="bc", name="bc")
                    nc.gpsimd.partition_broadcast(
                        out_ap=bc[:, :], in_ap=rec[0:1, :])
                    nc.vector.tensor_mul(
                        out=avT[po:po + 64, mc, q0:q0 + QB],
                        in0=avR[0:DH, :], in1=bc[:, :],
                    )

            # ---- startup: pair-0 k/q projection streams behind the hT DMA
            # (contraction-outer, accumulating into the two sc-pool tiles) ----
            kA = psc.tile([128, 2 * QB], f32, tag="sc", name="kA")
            kB = psc.tile([128, 2 * QB], f32, tag="sc", name="kB")
            qp = ppj.tile([128, QB], f32, tag="pj", name="qp0")
            for kc in range(KC):
                nc.tensor.matmul(
                    qp[:, :], wq_t0[:, kc, :], hT_sb[:, kc, 0:QB],
                    start=(kc == 0), stop=(kc == KC - 1),
                )
                for c2, t in ((0, kA), (1, kB)):
                    for n in (0, QB):
                        nc.tensor.matmul(
                            t[:, n:n + QB], wk_t0[:, kc, :],
                            hT_sb[:, kc, c2 * 1024 + n:c2 * 1024 + n + QB],
                            start=(kc == 0), stop=(kc == KC - 1),
                        )
            nc.vector.tensor_copy(out=kT[:, 0, 0:1024], in_=kA[:, :])
            nc.vector.tensor_copy(out=qT[:, 0, 0:QB], in_=qp[:, :])
            nc.vector.tensor_copy(out=kT[:, 0, 1024:2048], in_=kB[:, :])

            # ---- Pass 0 (query cols 0:512) + all projections ----
            with tc.tile_pool(name="pav", bufs=1) as pav:
                wv_sb = pav.tile([128, KC, D], fp16)
                nc.sync.dma_start(out=wv_sb[:, :, :], in_=wv[:, :, :])
                u0 = []
                for sc in range(SC):
                    u0 += vchunk(sc)
                u0 += kproj(1) + qproj(1, 0)
                attn(0, 0, units=u0)
                for mc in range(1, MC):
                    if mc < MC - 1:
                        un = kproj(mc + 1) + qproj(mc + 1, 0)
                    else:
                        un = qproj(0, 1)
                    attn(mc, 0, units=un)

            # ---- Pass 1 (query cols 512:1024) ----
            with (
                tc.tile_pool(name="pcw", bufs=1) as pcw,
                tc.tile_pool(name="pc", bufs=2) as pc,
                tc.tile_pool(name="pcs", bufs=2) as pcs,
            ):
                wo_sb = pcw.tile([128, KC, D], bf16)
                nc.sync.dma_start(out=wo_sb[:, :, :], in_=wo[:, :, :])
                hres_sb = pcw.tile([128, QC, D], fp16)
                for q in range(QC):
                    nc.sync.dma_start(out=hres_sb[:, q, :], in_=hres[:, q, :])
                if apply_gb:
                    gb_sb = pcw.tile([128, 2 * D], f32)
                    nc.gpsimd.dma_start(
                        out=gb_sb,
                        in_=bass.AP(tensor=gb, offset=0,
                                    ap=[[0, 128], [1, 2 * D]]),
                    )
                for mc in range(MC):
                    un = []
                    if 2 <= mc <= 5:
                        un += ochunk_early(mc - 2)
                    if mc < MC - 1:
                        un += qproj(mc + 1, 1)
                    attn(mc, 1, units=un or None)

                # ---- Phase C: finish early chunks, then remaining o-proj ----
                def ln_tail(q, stats_ap, x_ap):
                    mv = pcs.tile([128, 2], f32, tag="mv")
                    nc.vector.bn_aggr(out=mv[:, :], in_=stats_ap)
                    rstd = pcs.tile([128, 1], f32, tag="rstd")
                    sq = nc.scalar.activation(
                        out=rstd[:, :], in_=mv[:, 1:2],
                        func=mybir.ActivationFunctionType.Sqrt,
                        bias=eps_t[:, :], scale=1.0,
                    )
                    if last_exp[0] is not None:
                        add_dep_helper(sq.ins, last_exp[0].ins, sync=True,
                                       reason="sqrt after all exps (ACT table)")
                    nc.vector.reciprocal(out=rstd[:, :], in_=rstd[:, :])
                    y = pc.tile([128, D], f32, tag="y")
                    nc.vector.tensor_scalar(
                        out=y[:, :], in0=x_ap,
                        scalar1=mv[:, 0:1], scalar2=rstd[:, :],
                        op0=mybir.AluOpType.subtract,
                        op1=mybir.AluOpType.mult,
                    )
                    if apply_gb:
                        nc.vector.tensor_mul(
                            out=y[:, :], in0=y[:, :], in1=gb_sb[:, 0:D])
                        nc.vector.tensor_add(
                            out=y[:, :], in0=y[:, :], in1=gb_sb[:, D:2 * D])
                    (nc.sync if q % 2 == 0 else nc.scalar).dma_start(
                        out=out[:, q, :], in_=y[:, :])

                for q in range(4):
                    ln_tail(q, st_q[:, q, :, :], oq_sb[:, q, :])
                for q in range(4, QC):
                    o_ps = psc.tile([128, D], f32, tag="sc", name="ops")
                    for n in range(0, D, 512):
                        nc.tensor.matmul(
                            o_ps[:, n:n + 512], ident_sb[:, :],
                            hres_sb[:, q, n:n + 512],
                            start=True, stop=False,
                        )
                        for mc in range(MC):
                            nc.tensor.matmul(
                                o_ps[:, n:n + 512],
                                avT[:, mc, q * 128:(q + 1) * 128],
                                wo_sb[:, mc, n:n + 512],
                                start=False, stop=(mc == MC - 1),
                            )
                    st = pcs.tile([128, 2, 6], f32, tag="st")
                    nc.vector.bn_stats(out=st[:, 0, :], in_=o_ps[:, 0:512])
                    nc.vector.bn_stats(out=st[:, 1, :], in_=o_ps[:, 512:1024])
                    ln_tail(q, st[:, :, :], o_ps[:, :])

    nc.finalize()
    return nc


def _part_major(a: np.ndarray, chunks: int) -> np.ndarray:
    """[chunks*128, N] -> [128, chunks, N] (partition-major device layout)."""
    n = a.shape[1]
    return np.ascontiguousarray(a.reshape(chunks, 128, n).transpose(1, 0, 2))


def kernel(h, Wq, Wk, Wv, Wo, gamma, beta):
    h = np.asarray(h, dtype=np.float32)
    bf = ml_dtypes.bfloat16
    f16 = np.float16
    gamma = np.asarray(gamma, np.float32)
    beta = np.asarray(beta, np.float32)
    apply_gb = not (np.all(gamma == 1.0) and np.all(beta == 0.0))
    wq_d = _part_major(np.asarray(Wq).astype(f16), KC)
    wk_d = _part_major(np.asarray(Wk).astype(f16), KC)
    wv_d = _part_major(np.asarray(Wv).astype(f16), KC)
    wo_d = _part_major(np.asarray(Wo).astype(bf), KC)
    gb = np.concatenate([gamma, beta]).reshape(1, 2 * D)
    ident = np.eye(128, dtype=f16)

    in_maps = []
    for c in range(N_CORES):
        b, r = c // 2, (c % 2) * SQ
        # kv column rotation: this core's queries first (attention is
        # invariant to kv ordering; k/v are projected in the same order)
        hT_b = np.ascontiguousarray(
            np.roll(h[b], -r, axis=0).T).astype(f16)          # [D, S]
        in_maps.append({
            "hT": _part_major(hT_b, KC),
            "hres": _part_major(
                np.ascontiguousarray(h[b, r:r + SQ]).astype(f16), QC),
            "wq": wq_d, "wk": wk_d, "wv": wv_d, "wo": wo_d, "gb": gb,
            "ident": ident,
        })

    key = f"nc{int(apply_gb)}"
    if key not in _CACHE:
        _CACHE[key] = _build(apply_gb)
    res = run_bass_kernel_spmd(_CACHE[key], in_maps, core_ids=list(range(N_CORES)))
    _CACHE["last"] = res

    outp = np.empty((B, S, D), dtype=np.float32)
    for c in range(N_CORES):
        b, r = c // 2, (c % 2) * SQ
        o = res.results[c]["out"]  # [128, QC, D]
        outp[b, r:r + SQ] = o.transpose(1, 0, 2).reshape(SQ, D)
    return outp
